# revision 2
# baseline (speedup 1.0000x reference)
"""Criss-cross (CCNet) attention kernel for Trainium2, 8 NeuronCores — v2.

Sharding: core c in 0..7 -> batch b = c//2, value-channel half h = c%2.

v2 redesign vs baseline (325 us):
  - x cast to f16 on HOST: xin DMA halves to 16 MB/core.
  - all matmul operands 16-bit (f16 weights/x/QK, bf16 P/V): 1 cyc/row at
    any moving size (kills the fp32r 4x penalty on the 128-wide energy
    matmuls) and halves LDWEIGHTS traffic.
  - K is read as a partition-offset view QK[64:128] (no K2 copy).
  - v staging DRAM roundtrip in bf16 258-wide slots (16.9 MB vs 32 MB),
    ones columns stored with the slots so the col-pass gather needs no
    separate ones fill.
  - outputs are UNNORMALIZED aggregation sums in bf16 (O | rowsum) —
    no on-device reciprocal/scaling; host divides. 8.5 MB per pass.
  - PSUM: 2x[128,512] (row pqk/pE) + 1x[128,1024] (row pv / col pE) +
    1x[128,2048] 4-slot aggregation buffer = exactly 8 banks.
  - evac split Act/DVE so neither exceeds the PE's matmul time; col-pass
    aggregation is interleaved around the next sch's energy matmuls so
    the single psO buffer's evac latency hides.
"""

import numpy as np

import concourse.tile as tile
from concourse import bacc, mybir
from concourse.bass_utils import run_bass_kernel_spmd

B, C, H, W = 4, 512, 128, 128
CQK = C // 8          # 64
CV = C // 2           # 256 v channels per core
HW = H * W
N_CORES = 8

F32 = mybir.dt.float32
F16 = mybir.dt.float16
BF16 = mybir.dt.bfloat16
EXP = mybir.ActivationFunctionType.Exp
COPY = mybir.ActivationFunctionType.Copy

_CACHE = {}

VBW = 258   # v slot width: 256 channels + 2 ones columns

# K via partition-offset stationary QK[64:128] is ILLEGAL on TRN2
# ("Fmap and Weight must start at the same partition index") -> K2 copy
K_OFFSET = False


def _build(with_bias):
    nc = bacc.Bacc("TRN2", target_bir_lowering=False, debug=False,
                   num_devices=N_CORES)
    nck = 5 if with_bias else 4
    xrows = C + (2 if with_bias else 0)

    xin = nc.dram_tensor("xin", [xrows, HW], F16, kind="ExternalInput").ap()
    wqk = nc.dram_tensor("wqk", [xrows, 128], F16, kind="ExternalInput").ap()
    wv = nc.dram_tensor("wv", [xrows, CV], F16, kind="ExternalInput").ap()
    negid = nc.dram_tensor("negid", [128, 128], BF16,
                           kind="ExternalInput").ap()
    id4 = nc.dram_tensor("id4", [128, 512], BF16, kind="ExternalInput").ap()
    onesb = nc.dram_tensor("onesb", [128, 16], BF16,
                           kind="ExternalInput").ap()

    vscr = nc.dram_tensor("vscr", [HW, VBW], BF16).ap()
    orow = nc.dram_tensor("orow", [HW, VBW], BF16, kind="ExternalOutput").ap()
    ocol = nc.dram_tensor("ocol", [HW, VBW], BF16, kind="ExternalOutput").ap()

    with tile.TileContext(nc) as tc:
        with (
            tc.tile_pool(name="cst", bufs=1) as cst,
            tc.tile_pool(name="xs", bufs=4) as xsp,
            tc.tile_pool(name="p4", bufs=2) as p4p,
            tc.tile_pool(name="ob", bufs=2) as obp,
            tc.tile_pool(name="vc", bufs=3) as vcp,
            tc.tile_pool(name="psbig", bufs=2, space="PSUM") as psbigp,
            tc.tile_pool(name="psw", bufs=1, space="PSUM") as pswp,
            tc.tile_pool(name="psO", bufs=2, space="PSUM") as psOp,
        ):
            # prefetch first x chunks before constants so the first
            # projection matmuls start ASAP
            xpre = {}
            for ch0 in range(3):
                xt = xsp.tile([128, nck * 512], F16, tag="xs",
                              name=f"xpre{ch0}")
                nc.sync.dma_start(
                    xt[:].rearrange("p (k q) -> p k q", q=512)[:, 0:4, :],
                    xin.rearrange("(k p) q -> p k q", p=128)
                       [:, 0:4, ch0 * 512:(ch0 + 1) * 512])
                if with_bias:
                    nc.sync.dma_start(
                        xt[0:2, 4 * 512:5 * 512],
                        xin[512:514, ch0 * 512:(ch0 + 1) * 512])
                xpre[ch0] = xt

            # ---- persistent constants / state ----
            WQK = cst.tile([128, nck * 128], F16)
            for k in range(nck):
                rows = 128 if k < 4 else 2
                nc.sync.dma_start(WQK[0:rows, k * 128:k * 128 + 128],
                                  wqk[k * 128:k * 128 + rows, :])
            WV = cst.tile([128, nck * CV], F16)
            for k in range(nck):
                rows = 128 if k < 4 else 2
                nc.sync.dma_start(WV[0:rows, k * CV:k * CV + CV],
                                  wv[k * 128:k * 128 + rows, :])
            NEGID = cst.tile([128, 128], BF16)
            nc.sync.dma_start(NEGID[:], negid[:])
            ID4 = cst.tile([128, 512], BF16)
            nc.sync.dma_start(ID4[:], id4[:])

            QK = cst.tile([128, HW], F16)        # rows 0:64 q, 64:128 k
            if not K_OFFSET:
                K2 = cst.tile([64, HW], F16)
            # column-major copies of q/k: col-pass matmul operands must be
            # contiguous or the per-matmul LDWEIGHTS outruns the matmul and
            # the HAM clock-gate never ramps (measured: whole col pass at
            # ~1.0 GHz vs the row pass at ~2.2 GHz)
            KT = cst.tile([64, HW], F16)
            QT = cst.tile([64, HW], F16)

            # 8 rotating v slots [i, 258] with ones columns preloaded
            VB = cst.tile([128, 8 * VBW], BF16)
            vslots = VB[:].rearrange("p (s w) -> p s w", w=VBW)
            nc.sync.dma_start(vslots[:, :, 256:258],
                              onesb[:].rearrange("p (s w) -> p s w", w=2))

            vscr_row4 = vscr.rearrange("(g t x) c -> g x t c", t=4, x=128)
            vscr_col8 = vscr.rearrange("(j g t) c -> g j t c", t=8, j=128)
            # x-major orow rows (x*128+y): 4 consecutive DRAM rows per
            # partition -> 2 KB descriptors (516B ones run at half rate)
            orow_4 = orow.rearrange("(x g t) c -> g x t c", t=4, g=32)
            # y-major ocol rows (y*128+x): 8 consecutive rows -> 4 KB
            ocol_8 = ocol.rearrange("(y g t) c -> g y t c", t=8, g=16)

            def kslice(sl):
                return QK[64:128, sl] if K_OFFSET else K2[:, sl]

            mm_kw = {"tile_position": (0, 0)} if K_OFFSET else {}

            # =================== row pass ===================
            # software pipeline: chunk ch's aggregation tail is emitted
            # after chunk ch+1's head
            row_state = {}

            def load_x(ch):
                xt = xsp.tile([128, nck * 512], F16, tag="xs")
                nc.sync.dma_start(
                    xt[:].rearrange("p (k q) -> p k q", q=512)[:, 0:4, :],
                    xin.rearrange("(k p) q -> p k q", p=128)
                       [:, 0:4, ch * 512:(ch + 1) * 512])
                if with_bias:
                    nc.sync.dma_start(
                        xt[0:2, 4 * 512:5 * 512],
                        xin[512:514, ch * 512:(ch + 1) * 512])
                row_state[("xs", ch)] = xt

            def row_head_a(ch):
                csl = slice(ch * 512, (ch + 1) * 512)
                xt = row_state.pop(("xs", ch))
                # qk projection for these 512 pixels (4 rows)
                pqk = psbigp.tile([128, 512], F32, tag="psbig")
                for k in range(nck):
                    rows = 128 if k < 4 else 2
                    nc.tensor.matmul(pqk[:],
                                     WQK[0:rows, k * 128:(k + 1) * 128],
                                     xt[0:rows, k * 512:k * 512 + 512],
                                     start=(k == 0), stop=(k == nck - 1))
                nc.scalar.activation(QK[:, csl], pqk[:], COPY)
                if not K_OFFSET:
                    nc.vector.tensor_copy(K2[:, csl], QK[64:128, csl])
                # v projection: 4 row-tiles into one 2-bank psum
                pv = pswp.tile([128, 1024], F32, tag="psw")
                for yy in range(4):
                    for k in range(nck):
                        rows = 128 if k < 4 else 2
                        nc.tensor.matmul(
                            pv[:, yy * 256:(yy + 1) * 256],
                            xt[0:rows, k * 512 + yy * 128:
                               k * 512 + (yy + 1) * 128],
                            WV[0:rows, k * CV:(k + 1) * CV],
                            start=(k == 0), stop=(k == nck - 1))
                row_state[("pv", ch)] = pv

            def row_head_b(ch):
                csl = slice(ch * 512, (ch + 1) * 512)
                vbase = (ch % 2) * 4
                pv = row_state.pop(("pv", ch))
                # row energies E[i, x] for the 4 rows
                pE = psbigp.tile([128, 512], F32, tag="psbig")
                for yy in range(4):
                    y = ch * 4 + yy
                    ysl = slice(y * 128, (y + 1) * 128)
                    nc.tensor.matmul(pE[:, yy * 128:(yy + 1) * 128],
                                     kslice(ysl), QK[0:64, ysl],
                                     start=True, stop=True, **mm_kw)
                # column-major k/q slices for the col pass (on the
                # otherwise-idle gpsimd engine; SBUF->SBUF only)
                nc.gpsimd.tensor_copy(
                    KT[:].rearrange("c (x y) -> c x y", y=128)
                      [:, :, ch * 4:(ch + 1) * 4],
                    K2[:, csl].rearrange("c (y x) -> c x y", x=128))
                nc.gpsimd.tensor_copy(
                    QT[:].rearrange("c (x y) -> c x y", y=128)
                      [:, :, ch * 4:(ch + 1) * 4],
                    QK[0:64, csl].rearrange("c (y x) -> c x y", x=128))
                # evac v tiles to slots (bf16) and stage to DRAM
                nc.vector.tensor_copy(
                    vslots[:, vbase:vbase + 4, 0:256],
                    pv[:].rearrange("p (t c) -> p t c", c=256))
                nc.sync.dma_start(vscr_row4[ch],
                                  vslots[:, vbase:vbase + 4, :])
                p4 = p4p.tile([128, 512], BF16, tag="p4r")
                nc.scalar.activation(p4[:], pE[:], EXP)
                row_state[ch] = p4

            def row_tail(ch):
                vbase = (ch % 2) * 4
                p4 = row_state.pop(ch)
                ob = obp.tile([128, 4 * VBW], BF16, tag="obr")
                obs = ob[:].rearrange("p (t c) -> p t c", c=VBW)
                for half in range(2):
                    pO = psOp.tile([128, 1024], F32)
                    pOs = pO[:].rearrange("p (t c) -> p t c", c=512)
                    for q2 in range(2):
                        yy = half * 2 + q2
                        nc.tensor.matmul(
                            pO[:, q2 * 512:q2 * 512 + VBW],
                            p4[:, yy * 128:(yy + 1) * 128],
                            vslots[:, vbase + yy, :],
                            start=True, stop=True)
                    if half == 0:
                        nc.vector.tensor_copy(
                            obs[:, 0:2, :], pOs[:, :, 0:VBW])
                    else:
                        nc.scalar.activation(
                            obs[:, 2:4, :], pOs[:, :, 0:VBW], COPY)
                nc.sync.dma_start(orow_4[ch], obs)

            for ch0 in range(3):
                row_state[("xs", ch0)] = xpre[ch0]
            for ch in range(33):
                if ch < 32:
                    if ch + 3 < 32:
                        load_x(ch + 3)
                    row_head_a(ch)
                if ch >= 1:
                    row_tail(ch - 1)
                if ch < 32:
                    row_head_b(ch)

            # =================== column pass ===================
            # super-chunks of 8 columns; gathered v slots carry their own
            # ones columns. pE double-buffers through psbig (idle in this
            # pass) so energies never wait on the previous exp; the four
            # 2-column aggregation fills of sch-1 are spread between
            # sch's energy groups so psO evac latency hides under PE work.
            col_state = {}

            def col_gather(sch):
                vcb = vcp.tile([128, 8 * VBW], BF16, tag="vc")
                nc.sync.dma_start(
                    vcb[:].rearrange("j (x w) -> j x w", w=VBW),
                    vscr_col8[sch])
                col_state[("vc", sch)] = vcb

            def col_energy(sch, g):
                pE = psbigp.tile([128, 512], F32, tag="psbig")
                for xx in range(4):
                    x = sch * 8 + g * 4 + xx
                    xsl = slice(x * 128, (x + 1) * 128)
                    nc.tensor.matmul(pE[:, xx * 128:(xx + 1) * 128],
                                     KT[:, xsl], QT[:, xsl],
                                     start=(xx == 0), stop=False)
                nc.tensor.matmul(pE[:], NEGID[:], ID4[:],
                                 start=False, stop=True)
                if g == 0:
                    col_state[("p4", sch)] = p4p.tile(
                        [128, 1024], BF16, tag="p4c", name=f"p4c{sch}")
                p4 = col_state[("p4", sch)]
                nc.scalar.activation(p4[:, g * 512:(g + 1) * 512],
                                     pE[:], EXP)

            def col_agg(sch, q):
                # q in 0..3: aggregate columns sch*8+2q, +2q+1
                p4 = col_state[("p4", sch)]
                vcb = col_state[("vc", sch)]
                vcs = vcb[:].rearrange("j (x w) -> j x w", w=VBW)
                if q == 0:
                    col_state[("ob", sch)] = obp.tile(
                        [128, 8 * VBW], BF16, tag="obc", name=f"obc{sch}")
                oc = col_state[("ob", sch)]
                ocs = oc[:].rearrange("p (t c) -> p t c", c=VBW)
                pO = psOp.tile([128, 1024], F32)
                pOs = pO[:].rearrange("p (t c) -> p t c", c=512)
                for q2 in range(2):
                    xx = q * 2 + q2
                    nc.tensor.matmul(
                        pO[:, q2 * 512:q2 * 512 + VBW],
                        p4[:, xx * 128:(xx + 1) * 128],
                        vcs[:, xx, :],
                        start=True, stop=True)
                if q == 1:
                    nc.scalar.activation(ocs[:, 2:4, :],
                                         pOs[:, :, 0:VBW], COPY)
                else:
                    nc.vector.tensor_copy(ocs[:, q * 2:q * 2 + 2, :],
                                          pOs[:, :, 0:VBW])
                if q == 3:
                    nc.sync.dma_start(ocol_8[sch], ocs)
                    col_state.pop(("p4", sch))
                    col_state.pop(("vc", sch))
                    col_state.pop(("ob", sch))

            col_gather(0)
            col_gather(1)
            for sch in range(17):
                if sch < 16 and sch + 2 < 16:
                    col_gather(sch + 2)
                if sch >= 1:
                    col_agg(sch - 1, 0)
                if sch < 16:
                    col_energy(sch, 0)
                if sch >= 1:
                    col_agg(sch - 1, 1)
                    col_agg(sch - 1, 2)
                if sch < 16:
                    col_energy(sch, 1)
                if sch >= 1:
                    col_agg(sch - 1, 3)

    nc.compile()
    return nc


def _get_nc(with_bias):
    key = bool(with_bias)
    if key not in _CACHE:
        _CACHE[key] = _build(key)
    return _CACHE[key]


def kernel(x, Wq, bq, Wk, bk, Wv, bv, _trace=False, _raw=False):
    import ml_dtypes

    x = np.asarray(x, np.float32)
    Wq = np.asarray(Wq, np.float32)
    Wk = np.asarray(Wk, np.float32)
    Wv = np.asarray(Wv, np.float32)
    bq = np.asarray(bq, np.float32)
    bk = np.asarray(bk, np.float32)
    bv = np.asarray(bv, np.float32)

    with_bias = bool(np.any(bq) or np.any(bk) or np.any(bv))
    nc = _get_nc(with_bias)

    negid_a = np.ascontiguousarray(
        (-1e30 * np.eye(128)).astype(ml_dtypes.bfloat16))
    id4_a = np.ascontiguousarray(
        np.tile(np.eye(128), (1, 4)).astype(ml_dtypes.bfloat16))
    onesb = np.ones((128, 16), ml_dtypes.bfloat16)
    wqk_full = np.concatenate([Wq.T, Wk.T], axis=1)       # [C, 128]
    if with_bias:
        bias_qk = np.concatenate([bq, bk])[None, :]
        wqk_full = np.concatenate(
            [wqk_full, bias_qk, np.zeros_like(bias_qk)], axis=0)
    wqk_full = wqk_full.astype(np.float16)

    in_maps = []
    for core in range(N_CORES):
        b, h = core // 2, core % 2
        xb = np.ascontiguousarray(x[b].reshape(C, HW))
        wvh = np.ascontiguousarray(Wv[h * CV:(h + 1) * CV, :].T)  # [C, CV]
        if with_bias:
            xb = np.concatenate([xb, np.ones((1, HW), np.float32),
                                 np.zeros((1, HW), np.float32)], axis=0)
            bvh = bv[h * CV:(h + 1) * CV][None, :]
            wvh = np.concatenate([wvh, bvh, np.zeros_like(bvh)], axis=0)
        in_maps.append({
            "xin": xb.astype(np.float16),
            "wqk": wqk_full,
            "wv": wvh.astype(np.float16),
            "negid": negid_a, "id4": id4_a, "onesb": onesb,
        })

    res = run_bass_kernel_spmd(nc, in_maps, list(range(N_CORES)),
                               trace=bool(_trace))
    if _raw:
        return res

    out = np.empty((B, C, H, W), np.float32)
    for core in range(N_CORES):
        b, h = core // 2, core % 2
        r = res.results[core]
        o_r = r["orow"].astype(np.float32)     # x-major: [x*128+y, 258]
        o_c = r["ocol"].astype(np.float32)     # y-major: [y*128+x, 258]
        O_r = o_r[:, :256].reshape(W, H, CV).transpose(1, 0, 2)
        s_r = o_r[:, 256].reshape(W, H).T
        O_c = o_c[:, :256].reshape(H, W, CV)
        s_c = o_c[:, 256].reshape(H, W)
        comb = (O_r + O_c) / (s_r + s_c)[:, :, None]       # [y, x, c]
        out[b, h * CV:(h + 1) * CV] = comb.transpose(2, 0, 1)

    if _trace:
        return out, res
    return out
